# revision 44
# baseline (speedup 1.0000x reference)
"""Masked multi-head self-attention on 8 Trainium2 NeuronCores.

Sharding: core c handles batch b = c // 2 and head-group g = c % 2
(8 of 16 heads).  Data-parallel over B, tensor-parallel over heads for
qkv_proj (column split) / out_proj (row split).  The [T,T] causal mask
is exploited structurally (tile skipping); the host verifies the mask
is causal and falls back to numpy otherwise.  Host sums the two
head-group partial outputs per batch and adds bout.

Schedule: projection matmul groups are interleaved into the attention
stream so TensorE works through softmax (ScalarE) stretches, and the
softmax denominators are produced pre-broadcast across partitions by
using an all-ones [128,64] stationary in the row-sum matmuls.
"""

import numpy as np
import ml_dtypes

BF16 = ml_dtypes.bfloat16

B = 4
T = 2048
D = 1024
H = 16
DK = 64
P = 128
NCORES = 8

KT = D // P            # 8   k-tiles over d_model
TTILES = T // P        # 16  tiles over tokens
NCH = 4                # qi chunks of 512
CH = T // NCH          # 512

_CACHE = {}


def _build_program(with_bias=True):
    import concourse.bass as bass
    import concourse.tile as tile
    from concourse import bacc, mybir
    from contextlib import ExitStack

    f32 = mybir.dt.float32
    bf16 = mybir.dt.bfloat16
    nc = bacc.Bacc("TRN2", target_bir_lowering=False, debug=False,
                   num_devices=NCORES)

    xt_d = nc.declare_dram_parameter("xt", [P, KT * T], bf16, isOutput=False)
    wqk_d = nc.declare_dram_parameter("wqk", [P, 8 * 1024], bf16, isOutput=False)
    wv_d = nc.declare_dram_parameter("wv", [P, KT * 512], bf16, isOutput=False)
    wout_d = nc.declare_dram_parameter("wout", [P, 4 * 1024], bf16, isOutput=False)
    m01_d = nc.declare_dram_parameter("m01", [P, P], bf16, isOutput=False)
    bqk_d = nc.declare_dram_parameter("bqk", [P, 8], f32, isOutput=False)
    bv_d = nc.declare_dram_parameter("bv", [1, 512], bf16, isOutput=False)
    out_d = nc.declare_dram_parameter("out", [T, D], bf16, isOutput=True)

    ts = bass.ts
    EXP = mybir.ActivationFunctionType.Exp

    with tile.TileContext(nc) as tc, ExitStack() as top:
        const = top.enter_context(tc.tile_pool(name="const", bufs=1))
        big = top.enter_context(tc.tile_pool(name="big", bufs=1))
        wqk_pool = top.enter_context(tc.tile_pool(name="wqk", bufs=8))
        pt_pool = top.enter_context(tc.tile_pool(name="pt", bufs=10))
        rs_pool = top.enter_context(tc.tile_pool(name="rs", bufs=2))
        osb_pool = top.enter_context(tc.tile_pool(name="osb", bufs=4))
        # PSUM: "s" 2x[128,1024]f32 = 4 banks, "avr" 2x[128,512] = 2, "q" 2
        ps_s = top.enter_context(tc.tile_pool(name="ps_s", bufs=2, space="PSUM"))
        ps_avr = top.enter_context(tc.tile_pool(name="ps_avr", bufs=2, space="PSUM"))
        ps_q = top.enter_context(tc.tile_pool(name="ps_q", bufs=2, space="PSUM"))

        ones_row = const.tile([1, P], bf16, tag="ones_row")
        ones64 = const.tile([P, DK], bf16, tag="ones64")
        neg12 = const.tile([P, 1], f32, tag="neg12")
        bqk_sb = const.tile([P, 8], f32, tag="bqk")
        bv_sb = const.tile([1, 512], bf16, tag="bv")
        m01_blk = const.tile([P, P], bf16, tag="m01")
        nc.vector.memset(ones_row[:], 1.0)
        nc.vector.memset(ones64[:], 1.0)
        nc.vector.memset(neg12[:], -12.0)

        def qk_evac(dst, acc, i):
            if with_bias:
                nc.vector.tensor_scalar_add(dst, acc, bqk_sb[:, i:i + 1])
            else:
                nc.vector.tensor_copy(dst, acc)

        # qk[i] for i<4: q of head pair i (pre-scaled 1/8); i>=4: k of pair i-4
        xt = big.tile([P, KT * T], bf16, tag="xt")
        qk = [big.tile([P, T], bf16, tag=f"qk{i}", name=f"qk{i}")
              for i in range(8)]
        v_all = big.tile([P, TTILES * 512], bf16, tag="v")
        at_all = big.tile([P, 4 * T], bf16, tag="at")
        wv_sb = big.tile([P, KT * 512], bf16, tag="wv")
        wout_sb = big.tile([P, 4 * 1024], bf16, tag="wout")

        wqk_sb = {}

        def dma_wqk(i):
            w = wqk_pool.tile([P, 1024], bf16, tag="wqk", name=f"wqk{i}")
            nc.sync.dma_start(w[:], wqk_d[:, ts(i, 1024)])
            wqk_sb[i] = w

        if with_bias:
            nc.sync.dma_start(bqk_sb[:], bqk_d[:])
        dma_wqk(0)
        dma_wqk(4)
        # the prelude (qk chunk-0 + v tiles 0..3) reads only the first 512
        # cols of each k-block: stream those pieces first on both HWDGE
        # queues so compute starts earlier; everything the early fills need
        # (wv, wqk1/5, m01) is triggered before the xt remainders
        for kt in range(KT):
            eng = nc.scalar if kt % 2 == 0 else nc.sync
            eng.dma_start(xt[:, kt * T: kt * T + 512],
                          xt_d[:, kt * T: kt * T + 512])
        # wv in halves so the prelude v-groups start on kt 0-3 while 4-7 land
        nc.sync.dma_start(wv_sb[:, 0:4 * 512], wv_d[:, 0:4 * 512])
        nc.sync.dma_start(wv_sb[:, 4 * 512:], wv_d[:, 4 * 512:])
        if with_bias:
            nc.sync.dma_start(bv_sb[:], bv_d[:])
        dma_wqk(1)
        dma_wqk(5)
        nc.sync.dma_start(m01_blk[:], m01_d[:])
        for kt in range(KT):
            eng = nc.scalar if kt % 2 == 0 else nc.sync
            eng.dma_start(xt[:, kt * T + 512: (kt + 1) * T],
                          xt_d[:, kt * T + 512: (kt + 1) * T])
        nc.sync.dma_start(wout_sb[:], wout_d[:])
        dma_wqk(2)
        dma_wqk(6)
        dma_wqk(3)
        dma_wqk(7)

        def emit_qk_chunk(i, n, half=None, cell=None):
            """half=None: whole 8-matmul chain; half=0/1: split granule
            (the two granules share one psum tile via `cell`)."""
            if half in (None, 0):
                acc = ps_q.tile([P, 512], f32, tag="q", name="qkacc")
                if cell is not None:
                    cell.append(acc)
            else:
                acc = cell.pop()
            k0 = 0 if half in (None, 0) else KT // 2
            k1 = KT if half in (None, 1) else KT // 2
            for kt in range(k0, k1):
                nc.tensor.matmul(
                    acc[:], wqk_sb[i][:, ts(kt, P)],
                    xt[:, kt * T + n * CH: kt * T + (n + 1) * CH],
                    start=(kt == 0), stop=(kt == KT - 1))
            if half in (None, 1):
                qk_evac(qk[i][:, ts(n, CH)], acc[:], i)

        def emit_v_merged(t2):
            """v token-tiles 2*t2, 2*t2+1 in one 2-bank psum tile"""
            acc = ps_s.tile([P, 1024], f32, tag="s", name="vacc")
            for kt in range(KT):
                for h in range(2):
                    t = 2 * t2 + h
                    nc.tensor.matmul(
                        acc[:, ts(h, 512)],
                        xt[:, kt * T + t * P: kt * T + (t + 1) * P],
                        wv_sb[:, ts(kt, 512)],
                        start=(kt == 0),
                        stop=(not with_bias and kt == KT - 1))
            if with_bias:
                for h in range(2):
                    nc.tensor.matmul(acc[:, ts(h, 512)], ones_row[:],
                                     bv_sb[:], start=False, stop=True)
            nc.vector.tensor_copy(v_all[:, ts(t2, 1024)], acc[:])

        def emit_v_chunk(t, half=None, cell=None):
            if half in (None, 0):
                acc = ps_q.tile([P, 512], f32, tag="q", name="vacc")
                if cell is not None:
                    cell.append(acc)
            else:
                acc = cell.pop()
            k0 = 0 if half in (None, 0) else KT // 2
            k1 = KT if half in (None, 1) else KT // 2
            for kt in range(k0, k1):
                nc.tensor.matmul(
                    acc[:], xt[:, kt * T + t * P: kt * T + (t + 1) * P],
                    wv_sb[:, ts(kt, 512)],
                    start=(kt == 0),
                    stop=(not with_bias and kt == KT - 1))
            if half in (None, 1):
                if with_bias:
                    nc.tensor.matmul(acc[:], ones_row[:], bv_sb[:],
                                     start=False, stop=True)
                nc.vector.tensor_copy(v_all[:, ts(t, 512)], acc[:])

        def emit_op(t, dc):
            """out-projection for token tile t, output column half dc"""
            acc = ps_q.tile([P, 512], f32, tag="q", name="oacc")
            for kk in range(4):
                nc.tensor.matmul(
                    acc[:], at_all[:, kk * T + t * P: kk * T + (t + 1) * P],
                    wout_sb[:, kk * 1024 + dc * 512: kk * 1024 + dc * 512 + 512],
                    start=(kk == 0), stop=(kk == 3))
            o_sb = osb_pool.tile([P, 512], bf16, tag="o_sb")
            nc.vector.tensor_copy(o_sb[:], acc[:])
            nc.sync.dma_start(out_d[ts(t, P), ts(dc, 512)], o_sb[:])

        # ---- attention step machine ---------------------------------------
        # Globally software-pipelined: the scores for step i+1 (even across a
        # pair boundary) are issued while ScalarE computes exp(i), and fill
        # groups land inside the exp window, so neither engine waits.
        pair_ps = {}
        s_tiles = {}

        def pair_begin(c, p):
            rs_ps = ps_avr.tile([P, CH], f32, tag="avr", name="rs_ps")
            attn_ps = ps_avr.tile([P, CH], f32, tag="avr", name="attn_ps")
            pair_ps[(c, p)] = (rs_ps, attn_ps)

        def emit_score(c, p, j):
            kq = qk[4 + p]
            qq = qk[p]
            off = max(0, P * (j - 4 * c))
            s_ps = ps_s.tile([P, 1024], f32, tag="s", name="s_ps")
            nc.tensor.matmul(
                s_ps[:, off:512], kq[0:DK, ts(j, P)],
                qq[0:DK, c * CH + off:(c + 1) * CH],
                start=True, stop=True)
            nc.tensor.matmul(
                s_ps[:, 512 + off:1024], kq[DK:P, ts(j, P)],
                qq[DK:P, c * CH + off:(c + 1) * CH],
                start=True, stop=True)
            s_tiles[(c, p, j)] = s_ps

        def emit_exp(c, p, j):
            off = max(0, P * (j - 4 * c))
            s_ps = s_tiles.pop((c, p, j))
            pt = pt_pool.tile([P, 1024], bf16, tag="pt")
            # [128, 2, n] views pairing the two head-halves (stride 512),
            # so trimmed exp / diagonal masking are single instructions
            pt3 = pt[:].rearrange("p (two n) -> p two n", two=2)
            s3 = s_ps[:].rearrange("p (two n) -> p two n", two=2)
            m3 = m01_blk[:, None, :].broadcast_to([P, 2, P])
            if j > 4 * c:  # diagonal block, trimmed
                nc.scalar.activation(
                    pt3[:, :, off:512], s3[:, :, off:512],
                    EXP, bias=neg12[:], scale=1.0)
                nc.vector.tensor_mul(
                    pt3[:, :, off:off + P], pt3[:, :, off:off + P], m3)
            elif j == 4 * c:  # diagonal block at chunk start
                nc.scalar.activation(
                    pt[:], s_ps[:], EXP, bias=neg12[:], scale=1.0)
                nc.vector.tensor_mul(
                    pt3[:, :, 0:P], pt3[:, :, 0:P], m3)
            else:
                nc.scalar.activation(
                    pt[:], s_ps[:], EXP, bias=neg12[:], scale=1.0)
            return pt

        def emit_pv_rs(c, p, j, pt):
            rs_ps, attn_ps = pair_ps[(c, p)]
            nki = 4 * (c + 1)
            st = (j == 0)
            sp = (j == nki - 1)
            off = max(0, P * (j - 4 * c))
            vb = j * 512
            nc.tensor.matmul(
                attn_ps[0:DK, off:CH],
                v_all[:, vb + 2 * p * DK: vb + 2 * p * DK + DK],
                pt[:, off:512],
                start=st, stop=sp, skip_group_check=True)
            nc.tensor.matmul(
                attn_ps[DK:P, off:CH],
                v_all[:, vb + (2 * p + 1) * DK: vb + (2 * p + 2) * DK],
                pt[:, 512 + off:1024],
                start=st, stop=sp, skip_group_check=True)
            # denominators, replicated across partitions by the all-ones
            # [128,64] stationary: rows 0:64 <- sum(ptA), 64:128 <- sum(ptB)
            nc.tensor.matmul(
                rs_ps[0:DK, off:CH], ones64[:], pt[:, off:512],
                start=st, stop=sp, skip_group_check=True)
            nc.tensor.matmul(
                rs_ps[DK:P, off:CH], ones64[:], pt[:, 512 + off:1024],
                start=st, stop=sp, skip_group_check=True)

        def emit_pair_tail(c, p):
            rs_ps, attn_ps = pair_ps.pop((c, p))
            rs_sb = rs_pool.tile([P, CH], f32, tag="rs_sb")
            nc.vector.reciprocal_approx_fast(rs_sb[:], rs_ps[:])
            nc.vector.tensor_mul(
                at_all[:, p * T + c * CH: p * T + (c + 1) * CH],
                attn_ps[:], rs_sb[:])

        def qkf(i, n):
            """two ~1us granules sharing one psum accumulator"""
            cell = []
            return [lambda h=h: emit_qk_chunk(i, n, half=h, cell=cell)
                    for h in range(2)]

        def vf(t):
            cell = []
            return [lambda h=h: emit_v_chunk(t, half=h, cell=cell)
                    for h in range(2)]

        def opf(tt):
            return [lambda t=t, dc=dc: emit_op(t, dc)
                    for t in tt for dc in range(2)]

        # ---- prelude: only what pair (0,0) needs — chunk-0 of q0/k0 and
        # v tiles 0..3 (v accumulated in the idle "s" score pool) -----------
        emit_qk_chunk(0, 0)
        emit_qk_chunk(4, 0)
        emit_v_merged(0)
        emit_v_merged(1)

        # ---- main interleave.  qk projection chunk n of tile i is first
        # needed by pair (n, i%4), so each pair carries its successor's two
        # qk chunk-groups; v/out-proj groups fill the remaining slack, with
        # out-proj (no early deadline) pushed into the late exp-bound
        # chunks. ----------------------------------------------------------
        fills = {
            (0, 0): qkf(1, 0) + qkf(5, 0),
            (0, 1): qkf(2, 0) + qkf(6, 0),
            (0, 2): qkf(3, 0) + qkf(7, 0),
            (0, 3): qkf(0, 1) + qkf(4, 1) + vf(4) + vf(5) + vf(6) + vf(7),
            (1, 0): qkf(1, 1) + qkf(5, 1) + vf(8),
            (1, 1): qkf(2, 1) + qkf(6, 1) + vf(9),
            (1, 2): qkf(3, 1) + qkf(7, 1) + vf(10),
            (1, 3): qkf(0, 2) + qkf(4, 2) + vf(11),
            (2, 0): qkf(1, 2) + qkf(5, 2) + opf([0]),
            (2, 1): qkf(2, 2) + qkf(6, 2) + opf([1]),
            (2, 2): qkf(3, 2) + qkf(7, 2) + opf([2, 3]),
            (2, 3): qkf(0, 3) + qkf(4, 3) + vf(12) + vf(13) + opf([4]),
            (3, 0): vf(14) + vf(15) + qkf(1, 3) + qkf(5, 3) + opf([5]),
            (3, 1): qkf(2, 3) + qkf(6, 3) + opf([6, 7, 8]),
            (3, 2): qkf(3, 3) + qkf(7, 3) + opf([9, 10]),
            (3, 3): opf([11]),
        }
        steps = [(c, p, j)
                 for c in range(NCH) for p in range(4)
                 for j in range(4 * (c + 1))]
        pair_begin(0, 0)
        emit_score(0, 0, 0)
        fcur = {}
        for idx, (c, p, j) in enumerate(steps):
            nki = 4 * (c + 1)
            pt = emit_exp(c, p, j)
            if idx + 1 < len(steps):
                nc2, np2, nj2 = steps[idx + 1]
                if nj2 == 0:
                    pair_begin(nc2, np2)
                emit_score(nc2, np2, nj2)
            # fills must complete by step nki-2: the last step's lookahead
            # score reads qk chunks that this pair's fills produce
            fl = fills.get((c, p), ())
            want = min(len(fl), (j + 2) * len(fl) // nki)
            cur = fcur.get((c, p), 0)
            while cur < want:
                fl[cur]()
                cur += 1
            fcur[(c, p)] = cur
            emit_pv_rs(c, p, j, pt)
            if j == nki - 1:
                emit_pair_tail(c, p)
        for t in range(12, 16):
            for dc in range(2):
                emit_op(t, dc)

    nc.compile()
    return nc


def _get_program(with_bias):
    key = ("nc", with_bias)
    if key not in _CACHE:
        _CACHE[key] = _build_program(with_bias)
    return _CACHE[key]


def _prep_core_inputs(x, attn_mask, Wqkv, bqkv, Wout):
    """Per-core host-side sharding + DMA-friendly layouts."""
    # partial diagonal block: m01[ki_rel, qi_rel] = 1 iff qi_rel >= ki_rel
    m01 = np.triu(np.ones((P, P), np.float32)).astype(BF16)

    in_maps = []
    for core in range(NCORES):
        b, g = core // 2, core % 2
        xt = np.ascontiguousarray(
            x[b].T.reshape(KT, P, T).transpose(1, 0, 2).reshape(P, KT * T)
        ).astype(BF16)
        wq = Wqkv[:, 512 * g:512 * g + 512] * np.float32(0.125)
        wk = Wqkv[:, 1024 + 512 * g:1024 + 512 * g + 512]
        wqk = np.concatenate([wq, wk], axis=1)  # [1024, 1024]
        wqk = np.ascontiguousarray(
            wqk.reshape(KT, P, 8, P).transpose(1, 2, 0, 3).reshape(P, 8192)
        ).astype(BF16)
        wv = Wqkv[:, 2048 + 512 * g:2048 + 512 * g + 512]
        wv = np.ascontiguousarray(
            wv.reshape(KT, P, 512).transpose(1, 0, 2).reshape(P, KT * 512)
        ).astype(BF16)
        wo = Wout[512 * g:512 * g + 512, :]
        wo = np.ascontiguousarray(
            wo.reshape(4, P, 1024).transpose(1, 0, 2).reshape(P, 4096)
        ).astype(BF16)
        bq = bqkv[512 * g:512 * g + 512] * np.float32(0.125)
        bk = bqkv[1024 + 512 * g:1024 + 512 * g + 512]
        bqk = np.ascontiguousarray(
            np.concatenate([bq, bk]).reshape(8, P).T)
        bv = np.ascontiguousarray(
            bqkv[2048 + 512 * g:2048 + 512 * g + 512].reshape(1, 512)
        ).astype(BF16)
        in_maps.append({"xt": xt, "wqk": wqk, "wv": wv, "wout": wo,
                        "m01": m01, "bqk": bqk, "bv": bv})
    return in_maps


def _mask_is_causal(attn_mask):
    zero = (attn_mask == 0.0)
    if not np.array_equal(zero, np.tril(np.ones((T, T), dtype=bool))):
        return False
    return bool(np.all(attn_mask[~zero] <= np.float32(-50.0)))


def _numpy_fallback(x, attn_mask, Wqkv, bqkv, Wout, bout):
    qkv = x @ Wqkv + bqkv
    qkv = qkv.reshape(B, T, 3, H, DK).transpose(2, 0, 3, 1, 4)
    q, k, vv = qkv[0], qkv[1], qkv[2]
    scores = np.einsum("bhqd,bhkd->bhqk", q, k) / np.float32(np.sqrt(DK))
    scores = scores + attn_mask
    scores -= scores.max(axis=-1, keepdims=True)
    e = np.exp(scores)
    probs = e / e.sum(axis=-1, keepdims=True)
    attn = np.einsum("bhqk,bhkd->bhqd", probs, vv)
    attn = attn.transpose(0, 2, 1, 3).reshape(B, T, D)
    return (attn @ Wout + bout).astype(np.float32)


def _run(inputs, trace=False):
    from concourse.bass_utils import run_bass_kernel_spmd

    x = np.asarray(inputs["x"], dtype=np.float32)
    attn_mask = np.asarray(inputs["attn_mask"], dtype=np.float32)
    Wqkv = np.asarray(inputs["Wqkv"], dtype=np.float32)
    bqkv = np.asarray(inputs["bqkv"], dtype=np.float32)
    Wout = np.asarray(inputs["Wout"], dtype=np.float32)
    bout = np.asarray(inputs["bout"], dtype=np.float32)

    if not _mask_is_causal(attn_mask):
        return _numpy_fallback(x, attn_mask, Wqkv, bqkv, Wout, bout), None

    with_bias = bool(np.any(bqkv != 0.0))
    nc = _get_program(with_bias)
    in_maps = _prep_core_inputs(x, attn_mask, Wqkv, bqkv, Wout)
    res = run_bass_kernel_spmd(nc, in_maps, list(range(NCORES)), trace=trace)
    out = np.empty((B, T, D), np.float32)
    for b in range(B):
        out[b] = (res.results[2 * b]["out"].astype(np.float32)
                  + res.results[2 * b + 1]["out"].astype(np.float32) + bout)
    return out, res.exec_time_ns


def kernel(**inputs) -> np.ndarray:
    out, _ = _run(inputs, trace=False)
    return out
